# revision 43
# baseline (speedup 1.0000x reference)
"""Trainium2 Bass kernel for an 8-expert MoE layer (dense all-expert MLP).

Reference computation (B=8192 tokens, D=1024, E=8 experts, H=4096, top-2):
    logits = x @ Wg + bg;  scores = softmax(logits)
    top2 renormalized -> per-token weights for the 2 selected experts
    out = sum_e w_e * (relu(x @ W1[e] + b1[e]) @ W2[e] + b2[e])
    loss = 0.01 * sum(mean_tokens(scores)**2)

Sharding: expert-parallel across 8 NeuronCores. Core e holds W1[e]/W2[e]
(bf16, resident in SBUF) and streams all tokens through its expert in a
feature-major (transposed) dataflow, so weights are used as the stationary
matmul operand with no on-device transposes of activations. Gating must
match the fp32 reference's top-2 selection (bf16 logits would flip
near-ties), so x is shipped as bf16 + bf16 residual and reconstructed to
fp32 on-chip (exact to ~2^-17; validated 0 selection flips at 5.7x margin
on the worst token); the gate matmul then runs in fp32 against the fp32
gate weights. Gating is fused into the per-block main loop so all DMA
spreads across the whole kernel instead of a bandwidth-bound prologue.
Each core writes w_e * (expert_e out)^T; the host sums the 8 partials and
transposes back. The usage sums for the load-balance loss are computed
on-device (identical on every core; core 0's is used).
"""

import sys

for _p in ("/opt/trn_rl_repo", "/root/.axon_site/_ro/trn_rl_repo"):
    if _p not in sys.path:
        sys.path.append(_p)

import numpy as np
import ml_dtypes

import concourse.bass as bass
import concourse.mybir as mybir
import concourse.tile as tile
from concourse import bacc
import concourse.bass_utils as bass_utils
from concourse.bass import ts
from concourse.masks import make_identity

B, D, E, H = 8192, 1024, 8, 4096
DIVERSITY_PENALTY = 0.01
NCORES = 8
TB = 512            # token block (matmul moving free dim)
NB = B // TB        # 16 token blocks
KD = D // 128       # 8 contraction tiles over D
HT = H // 128       # 32 tiles over H
DT = D // 128       # 8 tiles over D (output)

F32 = mybir.dt.float32
BF16 = mybir.dt.bfloat16
AX = mybir.AxisListType
ALU = mybir.AluOpType
ACTF = mybir.ActivationFunctionType


def build_nc():
    nc = bacc.Bacc("TRN2", target_bir_lowering=False, debug=False,
                   num_devices=NCORES)

    xTbf = nc.dram_tensor("xTbf", [D, B], BF16, kind="ExternalInput")
    xTr = nc.dram_tensor("xTr", [D, B], BF16, kind="ExternalInput")
    w1 = nc.dram_tensor("w1", [D, H], BF16, kind="ExternalInput")
    w2 = nc.dram_tensor("w2", [H, D], BF16, kind="ExternalInput")
    b1v = nc.dram_tensor("b1v", [H], F32, kind="ExternalInput")
    b2v = nc.dram_tensor("b2v", [D], F32, kind="ExternalInput")
    wg = nc.dram_tensor("wg", [D, E], F32, kind="ExternalInput")
    bgv = nc.dram_tensor("bgv", [E], F32, kind="ExternalInput")
    esel = nc.dram_tensor("esel", [E], F32, kind="ExternalInput")
    outT = nc.dram_tensor("outT", [D, B], F32, kind="ExternalOutput")
    usage = nc.dram_tensor("usage", [1, E], F32, kind="ExternalOutput")

    with tile.TileContext(nc) as tc:
        with tc.tile_pool(name="consts", bufs=1) as cp, \
             tc.tile_pool(name="dram", bufs=1, space="DRAM") as dramp, \
             tc.tile_pool(name="xb", bufs=9) as xbp, \
             tc.tile_pool(name="xr", bufs=3) as xrp, \
             tc.tile_pool(name="xfr", bufs=2) as xfp, \
             tc.tile_pool(name="gsb", bufs=2) as gsb, \
             tc.tile_pool(name="glt", bufs=1) as gltp, \
             tc.tile_pool(name="hsb", bufs=1) as hp, \
             tc.tile_pool(name="osb", bufs=2) as osp, \
             tc.tile_pool(name="wbp", bufs=2) as wbp, \
             tc.tile_pool(name="pgx", bufs=2, space="PSUM") as pgxp, \
             tc.tile_pool(name="ph", bufs=4, space="PSUM") as php, \
             tc.tile_pool(name="po", bufs=2, space="PSUM") as pop:
            # Resident expert weights, partition-major per 128-row tile:
            # w1_sb[p, k, h] = W1[k*128+p, h]; lhsT slice = w1_sb[:, k, hj*128:...]
            # Loaded via the scalar engine's DMA queue, w1 split per k-tile,
            # so the first layer-1 matmuls start as soon as their chunk
            # lands and the x stream on nc.sync is not blocked.
            w1r = w1.ap().rearrange("(k p) h -> p k h", p=128)
            w1_sb = cp.tile([128, KD, H], BF16)
            for k in range(KD):
                nc.scalar.dma_start(out=w1_sb[:, k, :], in_=w1r[:, k, :])
            w2_sb = cp.tile([128, HT, D], BF16)
            nc.scalar.dma_start(out=w2_sb, in_=w2.ap().rearrange("(k p) d -> p k d", p=128))
            wg_sb = cp.tile([128, KD, E], F32)
            nc.gpsimd.dma_start(out=wg_sb, in_=wg.ap().rearrange("(k p) e -> p k e", p=128))
            # biases: column layouts (per-partition scalars)
            b1c = cp.tile([128, HT], F32)
            nc.gpsimd.dma_start(out=b1c, in_=b1v.ap().rearrange("(i p) -> p i", p=128))
            b2c = cp.tile([128, DT], F32)
            nc.gpsimd.dma_start(out=b2c, in_=b2v.ap().rearrange("(i p) -> p i", p=128))
            bg_col = cp.tile([E, 1], F32)
            _bg = bgv.ap()
            nc.gpsimd.dma_start(
                out=bg_col,
                in_=bass.AP(tensor=_bg.tensor, offset=_bg.offset, ap=[[1, E], [1, 1]]),
            )
            # esel broadcast across partitions: esel_b[p, e] = esel[e]
            _es = esel.ap()
            esel_b = cp.tile([128, E], F32)
            nc.gpsimd.dma_start(
                out=esel_b,
                in_=bass.AP(tensor=_es.tensor, offset=_es.offset, ap=[[0, 128], [1, E]]),
            )
            ident = cp.tile([128, 128], F32)
            make_identity(nc, ident)
            ones128 = cp.tile([128, 1], F32)
            nc.vector.memset(ones128, 1.0)
            usage_acc = cp.tile([128, E], F32)
            nc.vector.memset(usage_acc, 0.0)
            # DRAM staging for the per-token combine weight rows
            wrowd = dramp.tile([NB, TB], F32)
            wrowd_r = wrowd.rearrange("n (j p) -> n j p", j=4)

            for b in range(NB):
                # x streams: bf16 main (shared with layer 1) + bf16 residual;
                # fp32 gate operand reconstructed on-chip: xf = xb + xr.
                xbts = []
                for k in range(KD):
                    xb = xbp.tile([128, TB], BF16, tag="xb")
                    nc.sync.dma_start(out=xb, in_=xTbf.ap()[ts(k, 128), ts(b, TB)])
                    xbts.append(xb)
                pgT = pgxp.tile([E, TB], F32, tag="pgx")
                xrts = []
                for k in range(KD):
                    xr = xrp.tile([128, TB], BF16, tag="xr")
                    nc.sync.dma_start(out=xr, in_=xTr.ap()[ts(k, 128), ts(b, TB)])
                    xrts.append(xr)

                # layer 1: h^T = relu(W1^T x + b1), bf16, kept in SBUF.
                # The 8 fp32 gate matmuls ride along in the first 8 groups,
                # each consuming an xf tile reconstructed just-in-time, so
                # the PE never waits on the reconstruction.
                # Block 0 runs the gate ahead of layer 1: its matmuls only
                # need x tiles, covering the PE while the first w1 chunks
                # are still streaming in.
                if b == 0:
                    for k in range(KD):
                        xf = xfp.tile([128, TB], F32, tag="xf")
                        nc.gpsimd.tensor_add(xf, xbts[k], xrts[k])
                        nc.tensor.matmul(pgT, lhsT=wg_sb[:, k, :], rhs=xf,
                                         start=(k == 0), stop=(k == KD - 1),
                                         skip_group_check=True)
                h_sb = hp.tile([128, HT, TB], BF16)
                for i in range(HT):
                    ph = php.tile([128, TB], F32)
                    for k in range(KD):
                        nc.tensor.matmul(ph, lhsT=w1_sb[:, k, ts(i, 128)],
                                         rhs=xbts[k],
                                         start=(k == 0), stop=(k == KD - 1))
                    nc.scalar.activation(h_sb[:, i, :], ph, ACTF.Relu,
                                         bias=b1c[:, i:i + 1], scale=1.0)
                    if b > 0 and i < KD:
                        k = i
                        xf = xfp.tile([128, TB], F32, tag="xf")
                        nc.gpsimd.tensor_add(xf, xbts[k], xrts[k])
                        nc.tensor.matmul(pgT, lhsT=wg_sb[:, k, :], rhs=xf,
                                         start=(k == 0), stop=(k == KD - 1),
                                         skip_group_check=True)

                lT = gltp.tile([E, TB], F32)
                nc.scalar.activation(lT, pgT, ACTF.Identity, bias=bg_col, scale=1.0)
                wsel4 = gsb.tile([128, 4], F32, tag="wsel4", bufs=2)
                for j in range(TB // 128):
                    ltp = pgxp.tile([128, E], F32, tag="pgx")
                    nc.tensor.transpose(ltp, lT[:, ts(j, 128)], ident[:E, :E])
                    l = gsb.tile([128, E], F32)
                    nc.scalar.copy(l, ltp)
                    m8 = gsb.tile([128, 8], F32)
                    nc.vector.max(m8, l)
                    negm1 = gsb.tile([128, 1], F32)
                    nc.vector.tensor_scalar_mul(negm1, m8[:, 0:1], -1.0)
                    # p = exp(l - max) ; selection mask = (l >= 2nd max)
                    p = gsb.tile([128, E], F32)
                    nc.scalar.activation(p, l, ACTF.Exp, bias=negm1, scale=1.0)
                    mask = gsb.tile([128, E], F32)
                    nc.vector.tensor_scalar(mask, l, m8[:, 1:2], None, op0=ALU.is_ge)
                    wu = gsb.tile([128, E], F32)
                    nc.vector.tensor_mul(wu, p, mask)
                    den = gsb.tile([128, 1], F32)
                    nc.vector.reduce_sum(den, wu, axis=AX.X)
                    denr = gsb.tile([128, 1], F32)
                    nc.vector.reciprocal(denr, den)
                    wn = gsb.tile([128, E], F32)
                    nc.vector.tensor_scalar_mul(wn, wu, denr)
                    wsq = gsb.tile([128, E], F32)
                    nc.vector.tensor_mul(wsq, wn, esel_b)
                    nc.vector.reduce_sum(wsel4[:, j:j + 1], wsq, axis=AX.X)
                    # softmax scores for the usage/loss accumulator
                    z = gsb.tile([128, 1], F32)
                    nc.vector.reduce_sum(z, p, axis=AX.X)
                    zr = gsb.tile([128, 1], F32)
                    nc.vector.reciprocal(zr, z)
                    r = gsb.tile([128, E], F32)
                    nc.vector.tensor_scalar_mul(r, p, zr)
                    nc.vector.tensor_add(usage_acc, usage_acc, r)

                # Per-token weight columns -> one batched [128,4]->[4,128]
                # transpose -> straight to DRAM; the flat [4*128] layout is
                # exactly the token order the broadcast read expects.
                wrp4 = pgxp.tile([4, 128], F32, tag="pgx")
                nc.tensor.transpose(wrp4, wsel4, ident)
                wrow4 = gsb.tile([4, 128], F32, tag="wrow4", bufs=2)
                nc.scalar.copy(wrow4, wrp4)
                nc.sync.dma_start(out=wrowd_r[b, :, :], in_=wrow4)
                # combine weight broadcast to all 128 partitions
                wb = wbp.tile([128, TB], F32)
                nc.gpsimd.dma_start(
                    out=wb,
                    in_=bass.AP(tensor=wrowd.tensor, offset=wrowd.offset + b * TB,
                                ap=[[0, 128], [1, TB]]),
                )

                # layer 2: out^T = (W2^T h + b2) * w, accumulated over H
                for j in range(DT):
                    po = pop.tile([128, TB], F32)
                    for i in range(HT):
                        nc.tensor.matmul(po, lhsT=w2_sb[:, i, ts(j, 128)],
                                         rhs=h_sb[:, i, :],
                                         start=(i == 0), stop=(i == HT - 1))
                    ot = osp.tile([128, TB], F32)
                    nc.vector.scalar_tensor_tensor(ot, po, b2c[:, j:j + 1], wb,
                                                   op0=ALU.add, op1=ALU.mult)
                    nc.sync.dma_start(out=outT.ap()[ts(j, 128), ts(b, TB)], in_=ot)

            # usage sums over all tokens: ones^T @ usage_acc
            pu = pgxp.tile([1, E], F32, tag="pgx")
            nc.tensor.matmul(pu, lhsT=ones128, rhs=usage_acc, start=True, stop=True)
            us = gsb.tile([1, E], F32)
            nc.scalar.copy(us, pu)
            nc.sync.dma_start(out=usage.ap(), in_=us)

    nc.finalize()
    return nc


_NC_CACHE = {}


def _get_nc():
    if "nc" not in _NC_CACHE:
        _NC_CACHE["nc"] = build_nc()
    return _NC_CACHE["nc"]


def _enable_tracing():
    """Inject the missing antenv.axon_hooks + stub artifact upload so
    run_bass_kernel_spmd(trace=True) yields neuron-profile exec_time_ns."""
    import types
    if "antenv.axon_hooks" not in sys.modules:
        mod = types.ModuleType("antenv.axon_hooks")
        hook = [None]
        mod.set_axon_ntff_profile_hook = lambda h: hook.__setitem__(0, h)
        mod.get_axon_ntff_profile_hook = lambda: hook[0]
        sys.modules["antenv.axon_hooks"] = mod
        from trn_agent_boot.trn_boot import _ntff_profile_via_ctypes
        mod.set_axon_ntff_profile_hook(
            _ntff_profile_via_ctypes("/opt/axon/libaxon_pjrt.so"))
    bass_utils.upload_artifacts = lambda tmpdir: tmpdir


def _run(x, Wg, bg, W1, b1, W2, b2, trace=False):
    if trace:
        _enable_tracing()
    nc = _get_nc()

    x = np.asarray(x, dtype=np.float32)
    xT = np.ascontiguousarray(x.T)
    xTbf = xT.astype(ml_dtypes.bfloat16)
    xTr = (xT - xTbf.astype(np.float32)).astype(ml_dtypes.bfloat16)
    Wg = np.asarray(Wg, dtype=np.float32)
    bg = np.asarray(bg, dtype=np.float32)

    in_maps = []
    for e in range(NCORES):
        sel = np.zeros(E, np.float32)
        sel[e] = 1.0
        in_maps.append({
            "xTbf": xTbf,
            "xTr": xTr,
            "w1": np.ascontiguousarray(W1[e]).astype(ml_dtypes.bfloat16),
            "w2": np.ascontiguousarray(W2[e]).astype(ml_dtypes.bfloat16),
            "b1v": np.ascontiguousarray(b1[e]).astype(np.float32),
            "b2v": np.ascontiguousarray(b2[e]).astype(np.float32),
            "wg": Wg,
            "bgv": bg,
            "esel": sel,
        })

    # One retry: the axon-proxied execute can fail transiently with
    # NRT_EXEC_UNIT_UNRECOVERABLE right after another process released
    # the devices.
    try:
        res = bass_utils.run_bass_kernel_spmd(
            nc, in_maps, core_ids=list(range(NCORES)), trace=trace)
    except Exception:
        import time as _time
        _time.sleep(10)
        res = bass_utils.run_bass_kernel_spmd(
            nc, in_maps, core_ids=list(range(NCORES)), trace=trace)

    acc = res.results[0]["outT"].astype(np.float32, copy=True)
    for c in range(1, NCORES):
        acc += res.results[c]["outT"]
    out = np.ascontiguousarray(acc.T)

    usage_sums = res.results[0]["usage"].reshape(E).astype(np.float32)
    eu = usage_sums / np.float32(B)
    loss = np.float32(DIVERSITY_PENALTY) * np.sum(eu * eu, dtype=np.float32)
    return out, np.float32(loss), res


def kernel(x, Wg, bg, W1, b1, W2, b2):
    out, loss, _ = _run(x, Wg, bg, W1, b1, W2, b2, trace=False)
    return out, loss


# revision 49
# speedup vs baseline: 1.0037x; 1.0037x over previous
"""Trainium2 Bass kernel for an 8-expert MoE layer (dense all-expert MLP).

Reference computation (B=8192 tokens, D=1024, E=8 experts, H=4096, top-2):
    logits = x @ Wg + bg;  scores = softmax(logits)
    top2 renormalized -> per-token weights for the 2 selected experts
    out = sum_e w_e * (relu(x @ W1[e] + b1[e]) @ W2[e] + b2[e])
    loss = 0.01 * sum(mean_tokens(scores)**2)

Sharding: expert-parallel across 8 NeuronCores. Core e holds W1[e]/W2[e]
(bf16, resident in SBUF) and streams all tokens through its expert in a
feature-major (transposed) dataflow, so weights are used as the stationary
matmul operand with no on-device transposes of activations. Gating must
match the fp32 reference's top-2 selection (bf16 logits would flip
near-ties), so x is shipped as bf16 + bf16 residual and reconstructed to
fp32 on-chip (exact to ~2^-17; validated 0 selection flips at 5.7x margin
on the worst token); the gate matmul then runs in fp32 against the fp32
gate weights. Gating is fused into the per-block main loop so all DMA
spreads across the whole kernel instead of a bandwidth-bound prologue.
Each core writes w_e * (expert_e out)^T; the host sums the 8 partials and
transposes back. The usage sums for the load-balance loss are computed
on-device (identical on every core; core 0's is used).
"""

import sys

for _p in ("/opt/trn_rl_repo", "/root/.axon_site/_ro/trn_rl_repo"):
    if _p not in sys.path:
        sys.path.append(_p)

import numpy as np
import ml_dtypes

import concourse.bass as bass
import concourse.mybir as mybir
import concourse.tile as tile
from concourse import bacc
import concourse.bass_utils as bass_utils
from concourse.bass import ts
from concourse.masks import make_identity

B, D, E, H = 8192, 1024, 8, 4096
DIVERSITY_PENALTY = 0.01
NCORES = 8
TB = 512            # token block (matmul moving free dim)
NB = B // TB        # 16 token blocks
KD = D // 128       # 8 contraction tiles over D
HT = H // 128       # 32 tiles over H
DT = D // 128       # 8 tiles over D (output)

F32 = mybir.dt.float32
BF16 = mybir.dt.bfloat16
AX = mybir.AxisListType
ALU = mybir.AluOpType
ACTF = mybir.ActivationFunctionType


def build_nc():
    nc = bacc.Bacc("TRN2", target_bir_lowering=False, debug=False,
                   num_devices=NCORES)

    xTbf = nc.dram_tensor("xTbf", [D, B], BF16, kind="ExternalInput")
    xTr = nc.dram_tensor("xTr", [D, B], BF16, kind="ExternalInput")
    w1 = nc.dram_tensor("w1", [D, H], BF16, kind="ExternalInput")
    w2 = nc.dram_tensor("w2", [H, D], BF16, kind="ExternalInput")
    b1v = nc.dram_tensor("b1v", [H], F32, kind="ExternalInput")
    b2v = nc.dram_tensor("b2v", [D], F32, kind="ExternalInput")
    wg = nc.dram_tensor("wg", [D, E], F32, kind="ExternalInput")
    bgv = nc.dram_tensor("bgv", [E], F32, kind="ExternalInput")
    esel = nc.dram_tensor("esel", [E], F32, kind="ExternalInput")
    outT = nc.dram_tensor("outT", [D, B], F32, kind="ExternalOutput")
    usage = nc.dram_tensor("usage", [1, E], F32, kind="ExternalOutput")

    with tile.TileContext(nc) as tc:
        with tc.tile_pool(name="consts", bufs=1) as cp, \
             tc.tile_pool(name="dram", bufs=1, space="DRAM") as dramp, \
             tc.tile_pool(name="xb", bufs=10) as xbp, \
             tc.tile_pool(name="xr", bufs=4) as xrp, \
             tc.tile_pool(name="xfr", bufs=2) as xfp, \
             tc.tile_pool(name="gsb", bufs=2) as gsb, \
             tc.tile_pool(name="glt", bufs=1) as gltp, \
             tc.tile_pool(name="hsb", bufs=1) as hp, \
             tc.tile_pool(name="osb", bufs=3) as osp, \
             tc.tile_pool(name="wbp", bufs=2) as wbp, \
             tc.tile_pool(name="pgx", bufs=2, space="PSUM") as pgxp, \
             tc.tile_pool(name="ph", bufs=4, space="PSUM") as php, \
             tc.tile_pool(name="po", bufs=2, space="PSUM") as pop:
            # Resident expert weights, partition-major per 128-row tile:
            # w1_sb[p, k, h] = W1[k*128+p, h]; lhsT slice = w1_sb[:, k, hj*128:...]
            # Loaded via the scalar engine's DMA queue, w1 split per k-tile,
            # so the first layer-1 matmuls start as soon as their chunk
            # lands and the x stream on nc.sync is not blocked.
            w1r = w1.ap().rearrange("(k p) h -> p k h", p=128)
            w1_sb = cp.tile([128, KD, H], BF16)
            for k in range(KD):
                nc.scalar.dma_start(out=w1_sb[:, k, :], in_=w1r[:, k, :])
            w2_sb = cp.tile([128, HT, D], BF16)
            nc.scalar.dma_start(out=w2_sb, in_=w2.ap().rearrange("(k p) d -> p k d", p=128))
            wg_sb = cp.tile([128, KD, E], F32)
            nc.gpsimd.dma_start(out=wg_sb, in_=wg.ap().rearrange("(k p) e -> p k e", p=128))
            # biases: column layouts (per-partition scalars)
            b1c = cp.tile([128, HT], F32)
            nc.gpsimd.dma_start(out=b1c, in_=b1v.ap().rearrange("(i p) -> p i", p=128))
            b2c = cp.tile([128, DT], F32)
            nc.gpsimd.dma_start(out=b2c, in_=b2v.ap().rearrange("(i p) -> p i", p=128))
            bg_col = cp.tile([E, 1], F32)
            _bg = bgv.ap()
            nc.gpsimd.dma_start(
                out=bg_col,
                in_=bass.AP(tensor=_bg.tensor, offset=_bg.offset, ap=[[1, E], [1, 1]]),
            )
            # esel broadcast across partitions: esel_b[p, e] = esel[e]
            _es = esel.ap()
            esel_b = cp.tile([128, E], F32)
            nc.gpsimd.dma_start(
                out=esel_b,
                in_=bass.AP(tensor=_es.tensor, offset=_es.offset, ap=[[0, 128], [1, E]]),
            )
            ident = cp.tile([128, 128], F32)
            make_identity(nc, ident)
            ones128 = cp.tile([128, 1], F32)
            nc.vector.memset(ones128, 1.0)
            usage_acc = cp.tile([128, E], F32)
            nc.vector.memset(usage_acc, 0.0)
            # DRAM staging for the per-token combine weight rows
            wrowd = dramp.tile([NB, TB], F32)
            wrowd_r = wrowd.rearrange("n (j p) -> n j p", j=4)

            for b in range(NB):
                # x streams: bf16 main (shared with layer 1) + bf16 residual;
                # fp32 gate operand reconstructed on-chip: xf = xb + xr.
                xbts = []
                for k in range(KD):
                    xb = xbp.tile([128, TB], BF16, tag="xb")
                    nc.sync.dma_start(out=xb, in_=xTbf.ap()[ts(k, 128), ts(b, TB)])
                    xbts.append(xb)
                pgT = pgxp.tile([E, TB], F32, tag="pgx")
                xrts = []
                for k in range(KD):
                    xr = xrp.tile([128, TB], BF16, tag="xr")
                    nc.sync.dma_start(out=xr, in_=xTr.ap()[ts(k, 128), ts(b, TB)])
                    xrts.append(xr)

                # layer 1: h^T = relu(W1^T x + b1), bf16, kept in SBUF.
                # The 8 fp32 gate matmuls ride along in the first 8 groups,
                # each consuming an xf tile reconstructed just-in-time, so
                # the PE never waits on the reconstruction.
                # Block 0 runs the gate ahead of layer 1: its matmuls only
                # need x tiles, covering the PE while the first w1 chunks
                # are still streaming in.
                if b == 0:
                    for k in range(KD):
                        xf = xfp.tile([128, TB], F32, tag="xf")
                        nc.vector.tensor_add(xf, xbts[k], xrts[k])
                        nc.tensor.matmul(pgT, lhsT=wg_sb[:, k, :], rhs=xf,
                                         start=(k == 0), stop=(k == KD - 1),
                                         skip_group_check=True)
                h_sb = hp.tile([128, HT, TB], BF16)
                for i in range(HT):
                    ph = php.tile([128, TB], F32)
                    for k in range(KD):
                        nc.tensor.matmul(ph, lhsT=w1_sb[:, k, ts(i, 128)],
                                         rhs=xbts[k],
                                         start=(k == 0), stop=(k == KD - 1))
                    nc.scalar.activation(h_sb[:, i, :], ph, ACTF.Relu,
                                         bias=b1c[:, i:i + 1], scale=1.0)
                    if b > 0 and i < KD:
                        k = i
                        xf = xfp.tile([128, TB], F32, tag="xf")
                        nc.vector.tensor_add(xf, xbts[k], xrts[k])
                        nc.tensor.matmul(pgT, lhsT=wg_sb[:, k, :], rhs=xf,
                                         start=(k == 0), stop=(k == KD - 1),
                                         skip_group_check=True)

                lT = gltp.tile([E, TB], F32)
                nc.scalar.activation(lT, pgT, ACTF.Identity, bias=bg_col, scale=1.0)
                wsel4 = gsb.tile([128, 4], F32, tag="wsel4", bufs=2)
                for j in range(TB // 128):
                    ltp = pgxp.tile([128, E], F32, tag="pgx")
                    nc.tensor.transpose(ltp, lT[:, ts(j, 128)], ident[:E, :E])
                    l = gsb.tile([128, E], F32)
                    nc.scalar.copy(l, ltp)
                    m8 = gsb.tile([128, 8], F32)
                    nc.vector.max(m8, l)
                    negm1 = gsb.tile([128, 1], F32)
                    nc.vector.tensor_scalar_mul(negm1, m8[:, 0:1], -1.0)
                    # p = exp(l - max) ; selection mask = (l >= 2nd max)
                    p = gsb.tile([128, E], F32)
                    nc.scalar.activation(p, l, ACTF.Exp, bias=negm1, scale=1.0)
                    mask = gsb.tile([128, E], F32)
                    nc.vector.tensor_scalar(mask, l, m8[:, 1:2], None, op0=ALU.is_ge)
                    wu = gsb.tile([128, E], F32)
                    nc.vector.tensor_mul(wu, p, mask)
                    den = gsb.tile([128, 1], F32)
                    nc.vector.reduce_sum(den, wu, axis=AX.X)
                    denr = gsb.tile([128, 1], F32)
                    nc.vector.reciprocal(denr, den)
                    wn = gsb.tile([128, E], F32)
                    nc.vector.tensor_scalar_mul(wn, wu, denr)
                    wsq = gsb.tile([128, E], F32)
                    nc.vector.tensor_mul(wsq, wn, esel_b)
                    nc.vector.reduce_sum(wsel4[:, j:j + 1], wsq, axis=AX.X)
                    # softmax scores for the usage/loss accumulator
                    z = gsb.tile([128, 1], F32)
                    nc.vector.reduce_sum(z, p, axis=AX.X)
                    zr = gsb.tile([128, 1], F32)
                    nc.vector.reciprocal(zr, z)
                    r = gsb.tile([128, E], F32)
                    nc.vector.tensor_scalar_mul(r, p, zr)
                    nc.vector.tensor_add(usage_acc, usage_acc, r)

                # Per-token weight columns -> one batched [128,4]->[4,128]
                # transpose -> straight to DRAM; the flat [4*128] layout is
                # exactly the token order the broadcast read expects.
                wrp4 = pgxp.tile([4, 128], F32, tag="pgx")
                nc.tensor.transpose(wrp4, wsel4, ident)
                wrow4 = gsb.tile([4, 128], F32, tag="wrow4", bufs=2)
                nc.scalar.copy(wrow4, wrp4)
                nc.sync.dma_start(out=wrowd_r[b, :, :], in_=wrow4)
                # combine weight broadcast to all 128 partitions
                wb = wbp.tile([128, TB], F32)
                nc.gpsimd.dma_start(
                    out=wb,
                    in_=bass.AP(tensor=wrowd.tensor, offset=wrowd.offset + b * TB,
                                ap=[[0, 128], [1, TB]]),
                )

                # layer 2: out^T = (W2^T h + b2) * w, accumulated over H
                for j in range(DT):
                    po = pop.tile([128, TB], F32)
                    for i in range(HT):
                        nc.tensor.matmul(po, lhsT=w2_sb[:, i, ts(j, 128)],
                                         rhs=h_sb[:, i, :],
                                         start=(i == 0), stop=(i == HT - 1))
                    ot = osp.tile([128, TB], F32)
                    nc.vector.scalar_tensor_tensor(ot, po, b2c[:, j:j + 1], wb,
                                                   op0=ALU.add, op1=ALU.mult)
                    nc.sync.dma_start(out=outT.ap()[ts(j, 128), ts(b, TB)], in_=ot)

            # usage sums over all tokens: ones^T @ usage_acc
            pu = pgxp.tile([1, E], F32, tag="pgx")
            nc.tensor.matmul(pu, lhsT=ones128, rhs=usage_acc, start=True, stop=True)
            us = gsb.tile([1, E], F32)
            nc.scalar.copy(us, pu)
            nc.sync.dma_start(out=usage.ap(), in_=us)

    nc.finalize()
    return nc


_NC_CACHE = {}


def _get_nc():
    if "nc" not in _NC_CACHE:
        _NC_CACHE["nc"] = build_nc()
    return _NC_CACHE["nc"]


def _enable_tracing():
    """Inject the missing antenv.axon_hooks + stub artifact upload so
    run_bass_kernel_spmd(trace=True) yields neuron-profile exec_time_ns."""
    import types
    if "antenv.axon_hooks" not in sys.modules:
        mod = types.ModuleType("antenv.axon_hooks")
        hook = [None]
        mod.set_axon_ntff_profile_hook = lambda h: hook.__setitem__(0, h)
        mod.get_axon_ntff_profile_hook = lambda: hook[0]
        sys.modules["antenv.axon_hooks"] = mod
        from trn_agent_boot.trn_boot import _ntff_profile_via_ctypes
        mod.set_axon_ntff_profile_hook(
            _ntff_profile_via_ctypes("/opt/axon/libaxon_pjrt.so"))
    bass_utils.upload_artifacts = lambda tmpdir: tmpdir


def _run(x, Wg, bg, W1, b1, W2, b2, trace=False):
    if trace:
        _enable_tracing()
    nc = _get_nc()

    x = np.asarray(x, dtype=np.float32)
    xT = np.ascontiguousarray(x.T)
    xTbf = xT.astype(ml_dtypes.bfloat16)
    xTr = (xT - xTbf.astype(np.float32)).astype(ml_dtypes.bfloat16)
    Wg = np.asarray(Wg, dtype=np.float32)
    bg = np.asarray(bg, dtype=np.float32)

    in_maps = []
    for e in range(NCORES):
        sel = np.zeros(E, np.float32)
        sel[e] = 1.0
        in_maps.append({
            "xTbf": xTbf,
            "xTr": xTr,
            "w1": np.ascontiguousarray(W1[e]).astype(ml_dtypes.bfloat16),
            "w2": np.ascontiguousarray(W2[e]).astype(ml_dtypes.bfloat16),
            "b1v": np.ascontiguousarray(b1[e]).astype(np.float32),
            "b2v": np.ascontiguousarray(b2[e]).astype(np.float32),
            "wg": Wg,
            "bgv": bg,
            "esel": sel,
        })

    # One retry: the axon-proxied execute can fail transiently with
    # NRT_EXEC_UNIT_UNRECOVERABLE right after another process released
    # the devices.
    try:
        res = bass_utils.run_bass_kernel_spmd(
            nc, in_maps, core_ids=list(range(NCORES)), trace=trace)
    except Exception:
        import time as _time
        _time.sleep(10)
        res = bass_utils.run_bass_kernel_spmd(
            nc, in_maps, core_ids=list(range(NCORES)), trace=trace)

    acc = res.results[0]["outT"].astype(np.float32, copy=True)
    for c in range(1, NCORES):
        acc += res.results[c]["outT"]
    out = np.ascontiguousarray(acc.T)

    usage_sums = res.results[0]["usage"].reshape(E).astype(np.float32)
    eu = usage_sums / np.float32(B)
    loss = np.float32(DIVERSITY_PENALTY) * np.sum(eu * eu, dtype=np.float32)
    return out, np.float32(loss), res


def kernel(x, Wg, bg, W1, b1, W2, b2):
    out, loss, _ = _run(x, Wg, bg, W1, b1, W2, b2, trace=False)
    return out, loss
